# revision 14
# baseline (speedup 1.0000x reference)
"""Cost-volume kernel (nn_CostVolume) for Trainium2, 8 NeuronCores.

out[b, i, h, w] = mean_c feat1[b, c, h, w] * feat2[b, c, h, w + i - 4]
(feat2 zero-padded along width), inputs (8, 256, 96, 320) fp32,
output (8, 9, 96, 320) fp32.

Strategy
--------
Data-parallel over B: core b handles batch b (communication-free).

Per core, for each (h, 64-wide w-block) the 9 shifted channel-dot-products
are computed on the TensorEngine as a banded correlation matmul:

    band[p, n] = sum_c f1[c, w0+p] * f2[c, w0-4+n],   p in [0,64), n in [0,72)

with the C=256 contraction split into two PSUM-accumulated K=128 matmuls.
The 9 useful diagonals band[p, p+i] cannot be extracted by any lockstep
engine (per-partition-varying offsets).  Instead the bands are kept
SBUF-resident for all 96 h rows in (w-block, n, h) layout, written once to
an HBM scratch buffer (flat-addressed), and the diagonals are gathered by
DMA as 96-element contiguous h-runs with read stride (row+1) in (p, n).
The gathered [w, h] tiles are transposed to [h, w] on the TensorEngine and
written out contiguously.
"""

import numpy as np

import concourse.bacc as bacc
import concourse.bass as bass
import concourse.tile as tile
from concourse import mybir
from concourse.bass_utils import run_bass_kernel_spmd
from concourse.masks import make_identity

B, C, H, W = 8, 256, 96, 320
D = 4
NS = 2 * D + 1  # 9 shifts
P = 128  # partitions per c-block
M = 64  # w-block size
NB = W // M  # 5 w-blocks
NBAND = M + 2 * D  # 72 band columns
NH = 4  # h rows per feature chunk
NCHUNK = H // NH  # 24
WP = W + 2 * D  # padded feat2 row

F32 = mybir.dt.float32

_cache: dict = {}


def _build(reps: int = 1, skip_gather: bool = False, skip_compute: bool = False,
           skip_mm: bool = False, skip_act: bool = False):
    nc = bacc.Bacc("TRN2", target_bir_lowering=False, debug=False, num_devices=B)
    f1 = nc.dram_tensor("f1", (C, H, W), F32, kind="ExternalInput")
    f2 = nc.dram_tensor("f2", (C, H, W), F32, kind="ExternalInput")
    out = nc.dram_tensor("out", (NS, H, W), F32, kind="ExternalOutput")

    with tile.TileContext(nc) as tc:
        with (
            tc.tile_pool(name="consts", bufs=1) as cpool,
            tc.tile_pool(name="feat", bufs=2) as fpool,
            tc.tile_pool(name="band", bufs=1) as bpool,
            tc.tile_pool(name="gat", bufs=4) as gpool,
            tc.tile_pool(name="osb", bufs=2) as opool,
            tc.tile_pool(name="ps", bufs=6, space="PSUM") as pspool,
            tc.tile_pool(name="scratch", bufs=1, space="DRAM") as dpool,
        ):
            ident = cpool.tile([M, M], F32)
            make_identity(nc, ident)
            for _rep in range(reps):
                _body(
                    nc, tc, fpool, bpool, gpool, opool, pspool, dpool, ident,
                    f1, f2, out,
                    skip_gather=skip_gather, skip_compute=skip_compute,
                    skip_mm=skip_mm, skip_act=skip_act,
                )

    nc.compile()
    return nc


def _body(nc, tc, fpool, bpool, gpool, opool, pspool, dpool, ident, f1, f2, out,
          skip_gather=False, skip_compute=False, skip_mm=False, skip_act=False):
    # SBUF-resident bands for the full image: [p, blk, n, h].
    band = bpool.tile([M, NB, NBAND, H], F32, tag="band")

    for chunk in range(NCHUNK):
        h0 = chunk * NH
        f1t = []
        f2t = []
        for cb in range(2):
            t1 = fpool.tile([P, NH, W], F32, tag=f"f1_{cb}")
            nc.sync.dma_start(
                out=t1, in_=f1.ap()[cb * P : (cb + 1) * P, h0 : h0 + NH, :]
            )
            f1t.append(t1)
            t2 = fpool.tile([P, NH, WP], F32, tag=f"f2_{cb}")
            nc.vector.memset(t2[:, :, 0:D], 0.0)
            nc.vector.memset(t2[:, :, D + W : WP], 0.0)
            nc.sync.dma_start(
                out=t2[:, :, D : D + W],
                in_=f2.ap()[cb * P : (cb + 1) * P, h0 : h0 + NH, :],
            )
            f2t.append(t2)

        if skip_compute:
            continue
        for hl in range(NH):
            ps = pspool.tile([M, NB * NBAND], F32, tag="ps")
            if not skip_mm:
                for blk in range(NB):
                    w0 = blk * M
                    for cb in range(2):
                        nc.tensor.matmul(
                            ps[:, blk * NBAND : (blk + 1) * NBAND],
                            f1t[cb][:, hl, w0 : w0 + M],
                            f2t[cb][:, hl, w0 : w0 + NBAND],
                            start=(cb == 0),
                            stop=(cb == 1),
                        )
            if not skip_act:
                # psum (blk, n) -> band[:, blk, n, h0+hl], strided over n.
                # Alternate ACT/DVE so the two engines split the copy wall.
                if hl % 2 == 0:
                    nc.scalar.activation(
                        band[:, :, :, h0 + hl],
                        ps.rearrange("p (b n) -> p b n", b=NB),
                        mybir.ActivationFunctionType.Copy,
                        scale=1.0 / C,
                    )
                else:
                    nc.vector.tensor_scalar_mul(
                        band[:, :, :, h0 + hl],
                        ps.rearrange("p (b n) -> p b n", b=NB),
                        1.0 / C,
                    )

    if skip_compute or skip_gather:
        return

    # One contiguous dump of all bands to flat-addressed HBM scratch.
    scratch = dpool.tile([M, NB, NBAND, H], F32, tag="scr")
    nc.sync.dma_start(
        out=scratch.rearrange("p b n h -> p (b n h)"),
        in_=band.rearrange("p b n h -> p (b n h)"),
    )

    # Diagonal gathers (96-element h-runs), PE transpose, contiguous out.
    sc_p = NB * NBAND * H  # 34560: partition-row length in scratch
    with nc.allow_non_contiguous_dma("banded diagonal gather"):
        for i in range(NS):
            osb = opool.tile([H, W], F32, tag="osb")
            for blk in range(NB):
                g = gpool.tile([M, H], F32, tag="g")
                src = bass.AP(
                    tensor=scratch.tensor,
                    offset=scratch.offset + blk * NBAND * H + i * H,
                    ap=[[sc_p + H, M], [1, H]],
                )
                nc.sync.dma_start(out=g, in_=src)
                tp = pspool.tile([H, M], F32, tag="tp", bufs=2)
                nc.tensor.transpose(tp, g, ident)
                nc.scalar.copy(out=osb[:, blk * M : (blk + 1) * M], in_=tp)
            nc.sync.dma_start(out=out.ap()[i], in_=osb)


def kernel(feat1: np.ndarray, feat2: np.ndarray) -> np.ndarray:
    if "nc" not in _cache:
        _cache["nc"] = _build()
    nc = _cache["nc"]
    feat1 = np.ascontiguousarray(feat1, dtype=np.float32)
    feat2 = np.ascontiguousarray(feat2, dtype=np.float32)
    in_maps = [{"f1": feat1[b], "f2": feat2[b]} for b in range(B)]
    res = run_bass_kernel_spmd(nc, in_maps, core_ids=list(range(B)))
    return np.stack([res.results[b]["out"] for b in range(B)], axis=0)


# revision 28
# speedup vs baseline: 2.2112x; 2.2112x over previous
"""Cost-volume kernel (nn_CostVolume) for Trainium2, 8 NeuronCores.

out[b, i, h, w] = mean_c feat1[b, c, h, w] * feat2[b, c, h, w + i - 4]
(feat2 zero-padded along width), inputs (8, 256, 96, 320) fp32,
output (8, 9, 96, 320) fp32.

Strategy
--------
Data-parallel over B: core b handles batch b (communication-free).

Per core, for each (h, 64-wide w-block) the 9 shifted channel-dot-products
are computed on the TensorEngine as a banded correlation matmul:

    band[p, n] = sum_c f1[c, w0+p] * f2[c, w0-4+n],   p in [0,64), n in [0,72)

with the C=256 contraction split into two PSUM-accumulated K=128 matmuls.
The 9 useful diagonals band[p, p+i] cannot be extracted by any lockstep
engine (per-partition-varying offsets).  Instead the bands are kept
SBUF-resident for a 32-row h-slice in (w-block, n, h) layout, dumped
contiguously to flat-addressed HBM scratch, and the diagonals are gathered
back by DMA: columns [p, p+8] with h innermost are 9*32 consecutive
elements per partition-row, so one DMA per w-block with read stride
(row+1) and 1152-byte runs fetches every shift.  The gathered [w, (i, h)]
tiles are transposed to [h, w] on the TensorEngine (three shifts per
transpose) and written out contiguously.  Three h-slices pipeline the
dump/gather/output tail of one slice under the matmul loop of the next.

Measured (reps-slope on axon-tunneled trn2): ~0.10-0.13 ms per core,
~20x faster than the naive per-element diagonal-gather variant; the
kernel is input-DMA-bound (63 MB/core of fp32 features).
"""

import numpy as np

import concourse.bacc as bacc
import concourse.bass as bass
import concourse.tile as tile
from concourse import mybir
from concourse.bass_utils import run_bass_kernel_spmd
from concourse.masks import make_identity

B, C, H, W = 8, 256, 96, 320
D = 4
NS = 2 * D + 1  # 9 shifts
P = 128  # partitions per c-block
M = 64  # w-block size
NB = W // M  # 5 w-blocks
NBAND = M + 2 * D  # 72 band columns
NH = 8  # h rows per feature chunk
NCHUNK = H // NH  # 24
WP = W + 2 * D  # padded feat2 row
NHALF = 3  # image slices for tail pipelining (H2=32 keeps transpose
           # output slices 32-partition aligned)
H2 = H // NHALF  # h rows per half
CPH = NCHUNK // NHALF  # chunks per half

F32 = mybir.dt.float32

_cache: dict = {}


def _build(reps: int = 1, skip_gather: bool = False, skip_compute: bool = False,
           skip_mm: bool = False, skip_act: bool = False):
    nc = bacc.Bacc("TRN2", target_bir_lowering=False, debug=False, num_devices=B)
    f1 = nc.dram_tensor("f1", (C, H, W), F32, kind="ExternalInput")
    f2 = nc.dram_tensor("f2", (C, H, W), F32, kind="ExternalInput")
    out = nc.dram_tensor("out", (NS, H, W), F32, kind="ExternalOutput")

    with tile.TileContext(nc) as tc:
        with (
            tc.tile_pool(name="consts", bufs=1) as cpool,
            tc.tile_pool(name="feat", bufs=2) as fpool,
            tc.tile_pool(name="band", bufs=1) as bpool,
            tc.tile_pool(name="gat", bufs=4) as gpool,
            tc.tile_pool(name="osb", bufs=3) as opool,
            tc.tile_pool(name="ps", bufs=6, space="PSUM") as pspool,
            tc.tile_pool(name="scratch", bufs=1, space="DRAM") as dpool,
        ):
            ident = cpool.tile([M, M], F32)
            make_identity(nc, ident)
            pools = (fpool, bpool, gpool, opool, pspool, dpool)
            for _rep in range(reps):
                _body(
                    nc, tc, pools, ident, f1, f2, out,
                    skip_gather=skip_gather, skip_compute=skip_compute,
                    skip_mm=skip_mm, skip_act=skip_act,
                )

    nc.compile()
    return nc


def _tail(nc, pools, ident, out, band, half):
    """Dump one half's bands to HBM scratch, gather diagonals, transpose,
    and write out[._, half*H2:(half+1)*H2, :]."""
    fpool, bpool, gpool, opool, pspool, dpool = pools

    scratch = dpool.tile([M, NB, NBAND, H2], F32, tag=f"scr{half % 2}")
    for blk in range(NB):
        nc.sync.dma_start(
            out=scratch[:, blk].rearrange("p n h -> p (n h)"),
            in_=band[:, blk].rearrange("p n h -> p (n h)"),
        )

    # The 9 diagonals of row p are columns [p, p+8]: with h innermost these
    # are 9*H2 consecutive scratch elements per partition-row, so one DMA
    # per w-block gathers all shifts with (row+1)-strided 9*H2-element runs.
    sc_p = NB * NBAND * H2  # scratch partition-row length in elements
    g9s = []
    with nc.allow_non_contiguous_dma("banded diagonal gather"):
        for blk in range(NB):
            g9 = gpool.tile([M, NS, H2], F32, tag=f"g9_{blk}", bufs=1)
            src = bass.AP(
                tensor=scratch.tensor,
                offset=scratch.offset + blk * NBAND * H2,
                ap=[[sc_p + H2, M], [1, NS * H2]],
            )
            nc.sync.dma_start(out=g9.rearrange("p i h -> p (i h)"), in_=src)
            g9s.append(g9)
    # Transpose shift-triples [64, 3*H2] -> [3*H2, 64] in one PE op each,
    # splitting the psum->osb copies across ACT and DVE.
    for it in range(NS // 3):
        osbs = []
        for k in range(3):
            osb = opool.tile(
                [H2, W], F32, tag=f"osb{k}", bufs=2, name=f"osb_{it}_{k}"
            )
            osbs.append(osb)
        for blk in range(NB):
            tp = pspool.tile([3 * H2, M], F32, tag="tp", bufs=2)
            nc.tensor.transpose(
                tp, g9s[blk][:, 3 * it : 3 * it + 3, :].rearrange("p a b -> p (a b)"),
                ident,
            )
            for k in range(3):
                dst = osbs[k][:, blk * M : (blk + 1) * M]
                srcp = tp[k * H2 : (k + 1) * H2, :]
                if (blk + k) % 2 == 0:
                    nc.scalar.copy(out=dst, in_=srcp)
                else:
                    nc.vector.tensor_copy(dst, srcp)
        for k in range(3):
            i = 3 * it + k
            nc.sync.dma_start(
                out=out.ap()[i, half * H2 : (half + 1) * H2, :], in_=osbs[k]
            )


def _body(nc, tc, pools, ident, f1, f2, out,
          skip_gather=False, skip_compute=False, skip_mm=False, skip_act=False):
    fpool, bpool, gpool, opool, pspool, dpool = pools

    # Pre-allocated, manually double-buffered f2 tiles: the D-wide zero pads
    # are written once; chunk DMAs only touch the [D, D+W) interior.
    f2slots = [
        [
            fpool.tile(
                [P, NH, WP], F32, tag=f"f2_{cb}_{j}", bufs=1, name=f"f2s_{cb}_{j}"
            )
            for j in range(2)
        ]
        for cb in range(2)
    ]
    for cb in range(2):
        for j in range(2):
            nc.vector.memset(f2slots[cb][j][:, :, 0:D], 0.0)
            nc.vector.memset(f2slots[cb][j][:, :, D + W : WP], 0.0)

    for half in range(NHALF):
        # SBUF-resident bands for this slice: [p, blk, n, h2].
        band = bpool.tile([M, NB, NBAND, H2], F32, tag=f"band{half % 2}")

        for chunk in range(CPH):
            h0 = half * H2 + chunk * NH
            f1t = []
            f2t = []
            for cb in range(2):
                t1 = fpool.tile([P, NH, W], F32, tag=f"f1_{cb}")
                nc.sync.dma_start(
                    out=t1, in_=f1.ap()[cb * P : (cb + 1) * P, h0 : h0 + NH, :]
                )
                f1t.append(t1)
                t2 = f2slots[cb][(half * CPH + chunk) % 2]
                nc.sync.dma_start(
                    out=t2[:, :, D : D + W],
                    in_=f2.ap()[cb * P : (cb + 1) * P, h0 : h0 + NH, :],
                )
                f2t.append(t2)

            if skip_compute:
                continue
            for hl in range(NH):
                hloc = chunk * NH + hl  # h index within this half
                ps = pspool.tile([M, NB * NBAND], F32, tag="ps")
                if not skip_mm:
                    for blk in range(NB):
                        w0 = blk * M
                        for cb in range(2):
                            nc.tensor.matmul(
                                ps[:, blk * NBAND : (blk + 1) * NBAND],
                                f1t[cb][:, hl, w0 : w0 + M],
                                f2t[cb][:, hl, w0 : w0 + NBAND],
                                start=(cb == 0),
                                stop=(cb == 1),
                            )
                if not skip_act:
                    # psum (blk, n) -> band[:, blk, n, hloc], strided over n.
                    # Alternate ACT/DVE so two engines split the copy wall.
                    if hl % 2 == 0:
                        nc.scalar.activation(
                            band[:, :, :, hloc],
                            ps.rearrange("p (b n) -> p b n", b=NB),
                            mybir.ActivationFunctionType.Copy,
                            scale=1.0 / C,
                        )
                    else:
                        nc.vector.tensor_scalar_mul(
                            band[:, :, :, hloc],
                            ps.rearrange("p (b n) -> p b n", b=NB),
                            1.0 / C,
                        )

        if skip_compute or skip_gather:
            continue
        _tail(nc, pools, ident, out, band, half)


def kernel(feat1: np.ndarray, feat2: np.ndarray) -> np.ndarray:
    assert feat1.shape == (B, C, H, W), feat1.shape
    assert feat2.shape == (B, C, H, W), feat2.shape
    if "nc" not in _cache:
        _cache["nc"] = _build()
    nc = _cache["nc"]
    feat1 = np.ascontiguousarray(feat1, dtype=np.float32)
    feat2 = np.ascontiguousarray(feat2, dtype=np.float32)
    in_maps = [{"f1": feat1[b], "f2": feat2[b]} for b in range(B)]
    res = run_bass_kernel_spmd(nc, in_maps, core_ids=list(range(B)))
    return np.stack([res.results[b]["out"] for b in range(B)], axis=0)


# revision 35
# speedup vs baseline: 2.2548x; 1.0197x over previous
"""Cost-volume kernel (nn_CostVolume) for Trainium2, 8 NeuronCores.

out[b, i, h, w] = mean_c feat1[b, c, h, w] * feat2[b, c, h, w + i - 4]
(feat2 zero-padded along width), inputs (8, 256, 96, 320) fp32,
output (8, 9, 96, 320) fp32.

Strategy
--------
Data-parallel over B: core b handles batch b (communication-free).

Per core, for each (h, 64-wide w-block) the 9 shifted channel-dot-products
are computed on the TensorEngine as a banded correlation matmul:

    band[p, n] = sum_c f1[c, w0+p] * f2[c, w0-4+n],   p in [0,64), n in [0,72)

with the C=256 contraction split into two PSUM-accumulated K=128 matmuls.
The 9 useful diagonals band[p, p+i] cannot be extracted by any lockstep
engine (per-partition-varying offsets).  Instead the bands are kept
SBUF-resident for a 32-row h-slice in (w-block, n, h) layout, dumped
contiguously to flat-addressed HBM scratch, and the diagonals are gathered
back by DMA: columns [p, p+8] with h innermost are 9*32 consecutive
elements per partition-row, so one DMA per w-block with read stride
(row+1) and 1152-byte runs fetches every shift.  The gathered [w, (i, h)]
tiles are transposed to [h, w] on the TensorEngine (three shifts per
transpose) and written out contiguously.  Three h-slices pipeline the
dump/gather/output tail of one slice under the matmul loop of the next.

Measured (reps-slope on axon-tunneled trn2): ~0.08-0.13 ms per core,
~25x faster than the naive per-element diagonal-gather variant; the
kernel is input-DMA-bound (63 MB/core of fp32 features).  PSUM banks are
split 4/4 between matmul accumulation and tail transposes — starving the
transposes (6/2) costs ~45 us.
"""

import numpy as np

import concourse.bacc as bacc
import concourse.bass as bass
import concourse.tile as tile
from concourse import mybir
from concourse.bass_utils import run_bass_kernel_spmd
from concourse.masks import make_identity

B, C, H, W = 8, 256, 96, 320
D = 4
NS = 2 * D + 1  # 9 shifts
P = 128  # partitions per c-block
M = 64  # w-block size
NB = W // M  # 5 w-blocks
NBAND = M + 2 * D  # 72 band columns
NH = 8  # h rows per feature chunk
NCHUNK = H // NH  # 24
WP = W + 2 * D  # padded feat2 row
PS_BUFS = 4  # psum banks for matmul accumulation
TP_BUFS = 4  # psum banks for tail transposes
NHALF = 3  # image slices for tail pipelining (H2=32 keeps transpose
           # output slices 32-partition aligned)
H2 = H // NHALF  # h rows per half
CPH = NCHUNK // NHALF  # chunks per half

F32 = mybir.dt.float32

_cache: dict = {}


def _build(reps: int = 1, skip_gather: bool = False, skip_compute: bool = False,
           skip_mm: bool = False, skip_act: bool = False, ring_split: bool = False):
    nc = bacc.Bacc("TRN2", target_bir_lowering=False, debug=False, num_devices=B)
    f1 = nc.dram_tensor("f1", (C, H, W), F32, kind="ExternalInput")
    f2 = nc.dram_tensor("f2", (C, H, W), F32, kind="ExternalInput")
    out = nc.dram_tensor("out", (NS, H, W), F32, kind="ExternalOutput")

    with tile.TileContext(nc) as tc:
        with (
            tc.tile_pool(name="consts", bufs=1) as cpool,
            tc.tile_pool(name="feat", bufs=2) as fpool,
            tc.tile_pool(name="band", bufs=1) as bpool,
            tc.tile_pool(name="gat", bufs=4) as gpool,
            tc.tile_pool(name="osb", bufs=3) as opool,
            tc.tile_pool(name="ps", bufs=PS_BUFS, space="PSUM") as pspool,
            tc.tile_pool(name="scratch", bufs=1, space="DRAM") as dpool,
        ):
            ident = cpool.tile([M, M], F32)
            make_identity(nc, ident)
            pools = (fpool, bpool, gpool, opool, pspool, dpool)
            for _rep in range(reps):
                _body(
                    nc, tc, pools, ident, f1, f2, out,
                    skip_gather=skip_gather, skip_compute=skip_compute,
                    skip_mm=skip_mm, skip_act=skip_act, ring_split=ring_split,
                )

    nc.compile()
    return nc


def _tail(nc, pools, ident, out, band, half):
    """Dump one half's bands to HBM scratch, gather diagonals, transpose,
    and write out[._, half*H2:(half+1)*H2, :]."""
    fpool, bpool, gpool, opool, pspool, dpool = pools

    scratch = dpool.tile([M, NB, NBAND, H2], F32, tag=f"scr{half % 2}")
    for blk in range(NB):
        nc.sync.dma_start(
            out=scratch[:, blk].rearrange("p n h -> p (n h)"),
            in_=band[:, blk].rearrange("p n h -> p (n h)"),
        )

    # The 9 diagonals of row p are columns [p, p+8]: with h innermost these
    # are 9*H2 consecutive scratch elements per partition-row, so one DMA
    # per w-block gathers all shifts with (row+1)-strided 9*H2-element runs.
    sc_p = NB * NBAND * H2  # scratch partition-row length in elements
    g9s = []
    with nc.allow_non_contiguous_dma("banded diagonal gather"):
        for blk in range(NB):
            g9 = gpool.tile([M, NS, H2], F32, tag=f"g9_{blk}", bufs=1)
            src = bass.AP(
                tensor=scratch.tensor,
                offset=scratch.offset + blk * NBAND * H2,
                ap=[[sc_p + H2, M], [1, NS * H2]],
            )
            nc.sync.dma_start(out=g9.rearrange("p i h -> p (i h)"), in_=src)
            g9s.append(g9)
    # Transpose shift-triples [64, 3*H2] -> [3*H2, 64] in one PE op each,
    # splitting the psum->osb copies across ACT and DVE.
    for it in range(NS // 3):
        osbs = []
        for k in range(3):
            osb = opool.tile(
                [H2, W], F32, tag=f"osb{k}", bufs=2, name=f"osb_{it}_{k}"
            )
            osbs.append(osb)
        for blk in range(NB):
            tp = pspool.tile([3 * H2, M], F32, tag="tp", bufs=TP_BUFS)
            nc.tensor.transpose(
                tp, g9s[blk][:, 3 * it : 3 * it + 3, :].rearrange("p a b -> p (a b)"),
                ident,
            )
            for k in range(3):
                dst = osbs[k][:, blk * M : (blk + 1) * M]
                srcp = tp[k * H2 : (k + 1) * H2, :]
                if (blk + k) % 2 == 0:
                    nc.scalar.copy(out=dst, in_=srcp)
                else:
                    nc.vector.tensor_copy(dst, srcp)
        for k in range(3):
            i = 3 * it + k
            nc.sync.dma_start(
                out=out.ap()[i, half * H2 : (half + 1) * H2, :], in_=osbs[k]
            )


def _body(nc, tc, pools, ident, f1, f2, out,
          skip_gather=False, skip_compute=False, skip_mm=False, skip_act=False,
          ring_split=False):
    fpool, bpool, gpool, opool, pspool, dpool = pools
    # TRN2 has two HWDGE rings (SP + ACT); optionally split the two input
    # streams across them.
    dma2 = nc.scalar if ring_split else nc.sync

    # Pre-allocated, manually double-buffered f2 tiles: the D-wide zero pads
    # are written once; chunk DMAs only touch the [D, D+W) interior.
    f2slots = [
        [
            fpool.tile(
                [P, NH, WP], F32, tag=f"f2_{cb}_{j}", bufs=1, name=f"f2s_{cb}_{j}"
            )
            for j in range(2)
        ]
        for cb in range(2)
    ]
    for cb in range(2):
        for j in range(2):
            nc.vector.memset(f2slots[cb][j][:, :, 0:D], 0.0)
            nc.vector.memset(f2slots[cb][j][:, :, D + W : WP], 0.0)

    for half in range(NHALF):
        # SBUF-resident bands for this slice: [p, blk, n, h2].
        band = bpool.tile([M, NB, NBAND, H2], F32, tag=f"band{half % 2}")

        for chunk in range(CPH):
            h0 = half * H2 + chunk * NH
            f1t = []
            f2t = []
            for cb in range(2):
                t1 = fpool.tile([P, NH, W], F32, tag=f"f1_{cb}")
                nc.sync.dma_start(
                    out=t1, in_=f1.ap()[cb * P : (cb + 1) * P, h0 : h0 + NH, :]
                )
                f1t.append(t1)
                t2 = f2slots[cb][(half * CPH + chunk) % 2]
                dma2.dma_start(
                    out=t2[:, :, D : D + W],
                    in_=f2.ap()[cb * P : (cb + 1) * P, h0 : h0 + NH, :],
                )
                f2t.append(t2)

            if skip_compute:
                continue
            for hl in range(NH):
                hloc = chunk * NH + hl  # h index within this half
                ps = pspool.tile([M, NB * NBAND], F32, tag="ps")
                if not skip_mm:
                    for blk in range(NB):
                        w0 = blk * M
                        for cb in range(2):
                            nc.tensor.matmul(
                                ps[:, blk * NBAND : (blk + 1) * NBAND],
                                f1t[cb][:, hl, w0 : w0 + M],
                                f2t[cb][:, hl, w0 : w0 + NBAND],
                                start=(cb == 0),
                                stop=(cb == 1),
                            )
                if not skip_act:
                    # psum (blk, n) -> band[:, blk, n, hloc], strided over n.
                    # Alternate ACT/DVE so two engines split the copy wall.
                    if hl % 2 == 0:
                        nc.scalar.activation(
                            band[:, :, :, hloc],
                            ps.rearrange("p (b n) -> p b n", b=NB),
                            mybir.ActivationFunctionType.Copy,
                            scale=1.0 / C,
                        )
                    else:
                        nc.vector.tensor_scalar_mul(
                            band[:, :, :, hloc],
                            ps.rearrange("p (b n) -> p b n", b=NB),
                            1.0 / C,
                        )

        if skip_compute or skip_gather:
            continue
        _tail(nc, pools, ident, out, band, half)


def kernel(feat1: np.ndarray, feat2: np.ndarray) -> np.ndarray:
    assert feat1.shape == (B, C, H, W), feat1.shape
    assert feat2.shape == (B, C, H, W), feat2.shape
    if "nc" not in _cache:
        _cache["nc"] = _build()
    nc = _cache["nc"]
    feat1 = np.ascontiguousarray(feat1, dtype=np.float32)
    feat2 = np.ascontiguousarray(feat2, dtype=np.float32)
    in_maps = [{"f1": feat1[b], "f2": feat2[b]} for b in range(B)]
    res = run_bass_kernel_spmd(nc, in_maps, core_ids=list(range(B)))
    return np.stack([res.results[b]["out"] for b in range(B)], axis=0)
